# revision 49
# baseline (speedup 1.0000x reference)
"""DiffPool forward on 8 Trainium2 NeuronCores — one graph per core.

Per graph b (N=2048 nodes, F=C=K=128):
    fltr = D^-1/2 (A+I) D^-1/2,  Y = X @ [W_emb|W_pool]
    [Z | logits] = fltr @ Y      (d-scaling folded into matmul operands:
                                  psum = A @ (d*Y) + I @ (d*Y); out = d * psum)
    S = softmax(logits)
    X_pooled = S^T Z,  A_pooled = S^T (A S),  G = S^T S
    ||A - S S^T||_F^2 = sum(A) - 2 tr(A_pooled) + ||G||_F^2   (exact algebra —
                         the [N,N] S S^T is never materialized)
    entr = -sum(S log(S+eps))

A is binary {0,1} so it is sent to the device as bf16 losslessly (halves HBM
traffic); all matmuls run in bf16 with fp32 PSUM accumulation. Activations are
grouped by function across tiles to avoid ACT table reloads; rowsums are split
between DVE (reduce) and ACT (copy+accumulate) halves.
"""

import os
from contextlib import ExitStack

import numpy as np
import ml_dtypes

import concourse.bass as bass
import concourse.mybir as mybir
import concourse.tile as tile
from concourse import bacc
from concourse.bass import ts
from concourse.bass_utils import run_bass_kernel_spmd

F32 = mybir.dt.float32
BF16 = mybir.dt.bfloat16
AX = mybir.AxisListType.X
AF = mybir.ActivationFunctionType
OP = mybir.AluOpType

B, N, F, K, C = 8, 2048, 128, 128, 128
NT = N // 128  # 16 row-blocks
EPS = 1e-7
# phase-B processing order: drain psum pair-banks (t, t+8) early
PROC = [0, 8, 1, 9, 2, 10, 3, 11, 4, 12, 5, 13, 6, 14, 7, 15]


def _build(phases="D"):
    nc = bacc.Bacc(None, target_bir_lowering=False)
    lvl = {"0": 0, "A": 1, "B": 2, "D": 3}[phases]
    _run_phases(nc, lvl)
    if not nc.is_finalized():
        nc.finalize()
    return nc


def _run_phases(nc, lvl):
    A_d = nc.dram_tensor("A", [N, N], BF16, kind="ExternalInput")
    XT_d = nc.dram_tensor("XT", [F, N], BF16, kind="ExternalInput")
    W_d = nc.dram_tensor("W", [F, 2 * C], BF16, kind="ExternalInput")
    EYE_d = nc.dram_tensor("EYE", [128, 128], BF16, kind="ExternalInput")

    S_d = nc.dram_tensor("S", [N, K], F32, kind="ExternalOutput")
    XP_d = nc.dram_tensor("XP", [K, C], F32, kind="ExternalOutput")
    AP_d = nc.dram_tensor("AP", [K, K], F32, kind="ExternalOutput")
    AUX_d = nc.dram_tensor("AUX", [128, 3], F32, kind="ExternalOutput")

    with tile.TileContext(nc) as tc, ExitStack() as ctx:
        per = ctx.enter_context(tc.tile_pool(name="per", bufs=1))
        tmp = ctx.enter_context(tc.tile_pool(name="tmp", bufs=3))
        sml = ctx.enter_context(tc.tile_pool(name="sml", bufs=4))
        ps = ctx.enter_context(tc.tile_pool(name="ps", bufs=1, space="PSUM"))

        # ---- phase 0: X^T, W, EYE in; Y = X @ [W_emb|W_pool] per block ----
        a16 = per.tile([128, NT, N], BF16, tag="a16")
        xt16 = per.tile([128, N], BF16, tag="xt16")
        nc.sync.dma_start(out=xt16, in_=XT_d[:, :])
        w16 = per.tile([128, 2 * C], BF16, tag="w16")
        nc.sync.dma_start(out=w16, in_=W_d[:, :])
        eye16 = per.tile([128, 128], BF16, tag="eye16")
        nc.sync.dma_start(out=eye16, in_=EYE_d[:, :])

        y32 = per.tile([128, NT, 2 * C], F32, tag="y32")
        for j in range(NT):
            yp = ps.tile([128, 2 * C], F32, tag=f"b{6 + j % 2}", name=f"yp{j}")
            nc.tensor.matmul(yp, xt16[:, ts(j, 128)], w16, start=True, stop=True)
            nc.vector.tensor_copy(out=y32[:, j, :], in_=yp)

        if lvl < 1:
            return
        # ---- phase A: stream A row-blocks; rowsum -> d; pass-1 matmuls ----
        raD = per.tile([128, NT], F32, tag="raD")
        raA = per.tile([128, NT], F32, tag="raA")
        rsum = per.tile([128, NT], F32, tag="rsum")
        sq_all = per.tile([128, NT], F32, tag="sq")
        d_all = per.tile([128, NT], F32, tag="d")
        ent_all = per.tile([128, NT], F32, tag="ent")

        accs = [ps.tile([128, 512], F32, tag=f"b{t}", name=f"acc{t}")
                for t in range(8)]

        def acc_region(i):
            return accs[i % 8][:, (i // 8) * 256:(i // 8) * 256 + 256]

        # PSUM start/stop is bank-granular: each [128,512] bank (holding the
        # accumulators for output tiles t and t+8) is ONE accumulation group
        # of 34 matmuls (2*16 A-blocks + 2 self-loop identities).
        n_mm = [0] * 8
        BANK_MMS = 34
        HALF = N // 2
        for j in range(NT):
            nc.sync.dma_start(out=a16[:, j, :], in_=A_d[ts(j, 128), :])
            # rowsum(A), split ACT/DVE (exact: binary summands into fp32)
            rscr = tmp.tile([128, 1280], BF16, tag="rscr")
            nc.scalar.activation(out=rscr, in_=a16[:, j, 0:1280],
                                 func=AF.Copy, accum_out=raA[:, j:j + 1])
            nc.vector.reduce_sum(out=raD[:, j:j + 1],
                                 in_=a16[:, j, 1280:2048], axis=AX)
            nc.vector.tensor_add(rsum[:, j:j + 1], raD[:, j:j + 1],
                                 raA[:, j:j + 1])
            # d = 1/sqrt(rowsum + 1)
            nc.scalar.activation(out=sq_all[:, j:j + 1], in_=rsum[:, j:j + 1],
                                 func=AF.Sqrt, bias=1.0, scale=1.0)
            nc.vector.reciprocal(out=d_all[:, j:j + 1], in_=sq_all[:, j:j + 1])
            # yhat_j = d_j * Y_j   (bf16)
            yh = tmp.tile([128, 2 * C], BF16, tag="yhat")
            nc.vector.tensor_scalar_mul(out=yh, in0=y32[:, j, :],
                                        scalar1=d_all[:, j:j + 1])
            # banks 6/7 last: their PSUM slots are freed by the phase-0
            # Y-psum drains, so ordering them last keeps the in-order PE
            # queue from stalling on them at the start of block 0
            for i in (0, 1, 2, 3, 4, 5, 8, 9, 10, 11, 12, 13, 6, 14, 7, 15):
                t = i % 8
                nc.tensor.matmul(acc_region(i), a16[:, j, ts(i, 128)], yh,
                                 start=(n_mm[t] == 0),
                                 stop=(n_mm[t] == BANK_MMS - 1))
                n_mm[t] += 1
                if i == j:  # self-loop: psum_i += I^T @ yhat_i
                    nc.tensor.matmul(acc_region(i), eye16, yh,
                                     start=(n_mm[t] == 0),
                                     stop=(n_mm[t] == BANK_MMS - 1))
                    n_mm[t] += 1

        if lvl < 2:
            return
        # ---- phase B: softmax / entropy / Z scaling / S out / XG matmul ----
        # Structured as per-function passes over all 16 tiles so the ACT
        # engine loads each activation table exactly once.
        s32 = per.tile([128, NT, K], F32, tag="s32")      # S in fp32 (DMA out)
        zs16 = per.tile([128, NT, 2 * C], BF16, tag="zs16")  # [Z | S] bf16
        eps_t = per.tile([128, 1], F32, tag="eps")
        nc.vector.memset(eps_t, EPS)

        sumexps = per.tile([128, NT], F32, tag="sumexps")
        rexps = per.tile([128, NT], F32, tag="rexps")

        xg = ps.tile([128, 256], F32, tag="b0")  # bank 0 after drain
        # per-tile chains (all ACT work here is Exp -> one table load);
        # consecutive tiles pipeline across DVE/ACT/PE. No max-subtraction:
        # logits = d * psum with |logit| < 1 (d <= 1/sqrt(deg) ~ 1/14), so
        # exp cannot overflow, and softmax is shift-invariant.
        for step, i in enumerate(PROC):
            # E = exp(d*logits), rowsum -> sumexp
            nc.scalar.activation(out=s32[:, i, :], in_=acc_region(i)[:, 128:256],
                                 func=AF.Exp, bias=0.0,
                                 scale=d_all[:, i:i + 1],
                                 accum_out=sumexps[:, i:i + 1])
            nc.vector.reciprocal(out=rexps[:, i:i + 1], in_=sumexps[:, i:i + 1])
            nc.vector.tensor_scalar_mul(out=s32[:, i, :], in0=s32[:, i, :],
                                        scalar1=rexps[:, i:i + 1])
            nc.vector.tensor_copy(out=zs16[:, i, 128:256], in_=s32[:, i, :])
            nc.vector.tensor_scalar_mul(out=zs16[:, i, 0:128],
                                        in0=acc_region(i)[:, 0:128],
                                        scalar1=d_all[:, i:i + 1])
            # [X_pooled | G] += S_i^T @ [Z_i | S_i]   (bf16)
            nc.tensor.matmul(xg, zs16[:, i, 128:256], zs16[:, i, :],
                             start=(step == 0), stop=(step == NT - 1))

        # XP and ||G||^2 as soon as the XG accumulation stops (overlaps C)
        aux = per.tile([128, 3], F32, tag="aux")
        xp_out = per.tile([128, 128], F32, tag="xpo")
        nc.vector.tensor_copy(out=xp_out, in_=xg[:, 0:128])
        nc.sync.dma_start(out=XP_d[:, :], in_=xp_out)
        gscr = per.tile([128, 128], F32, tag="gscr")
        nc.scalar.activation(out=gscr, in_=xg[:, 128:256], func=AF.Square,
                             accum_out=aux[:, 2:3])
        nc.vector.reduce_sum(out=aux[:, 0:1], in_=rsum, axis=AX)

        if lvl < 3:
            return
        # ---- phase C: T = A @ S (bf16), A_pooled = S^T T ----
        tps = [ps.tile([128, 512], F32, tag=f"b{1 + q}", name=f"tp{q}")
               for q in range(4)]

        def t_region(i):
            return tps[i // 4][:, (i % 4) * 128:(i % 4) * 128 + 128]

        # Bank-major: finish one tp bank's 64-matmul group, then drain it
        # (ACT bf16 copy + A_pooled matmuls) while the next bank accumulates.
        ap_ps = ps.tile([128, 128], F32, tag="b5")
        for q in range(4):
            for idx, j in enumerate(PROC):
                for r in range(4):
                    i = 4 * q + r
                    nc.tensor.matmul(t_region(i), a16[:, j, ts(i, 128)],
                                     zs16[:, j, 128:256],
                                     start=(idx == 0 and r == 0),
                                     stop=(idx == NT - 1 and r == 3))
            for r in range(4):
                i = 4 * q + r
                t16 = tmp.tile([128, 128], BF16, tag="t16", bufs=4,
                               name=f"t16_{i}")
                nc.vector.tensor_copy(out=t16, in_=t_region(i))
                nc.tensor.matmul(ap_ps, zs16[:, i, 128:256], t16,
                                 start=(i == 0), stop=(i == NT - 1))

        # S out + entropy (off the critical path; only feeds S_d and AUX)
        for i in PROC:
            nc.sync.dma_start(out=S_d[ts(i, 128), :], in_=s32[:, i, :])
        for i in PROC:  # ACT: ln(S+eps); DVE: entropy partial
            l32 = tmp.tile([128, 128], F32, tag="l32")
            nc.scalar.activation(out=l32, in_=s32[:, i, :], func=AF.Ln,
                                 bias=eps_t, scale=1.0)
            escr = tmp.tile([128, 128], F32, tag="escr")
            nc.vector.tensor_mul(escr, s32[:, i, :], l32)
            nc.vector.reduce_sum(out=ent_all[:, i:i + 1], in_=escr,
                                 axis=AX)

        # ---- phase D: remaining outputs ----
        ap_out = per.tile([128, 128], F32, tag="apo")
        nc.vector.tensor_copy(out=ap_out, in_=ap_ps)
        nc.sync.dma_start(out=AP_d[:, :], in_=ap_out)
        nc.vector.reduce_sum(out=aux[:, 1:2], in_=ent_all, axis=AX)
        nc.sync.dma_start(out=AUX_d[:, :], in_=aux)


_NC = None


def _get_nc():
    global _NC
    if _NC is None:
        _NC = _build(os.environ.get("DIFFPOOL_PHASES", "D"))
    return _NC


def run(X, A, W_emb, W_pool, trace=False):
    nc = _get_nc()
    W = np.concatenate([np.asarray(W_emb, np.float32),
                        np.asarray(W_pool, np.float32)], axis=1)
    W = W.astype(ml_dtypes.bfloat16)
    eye = np.eye(128, dtype=np.float32).astype(ml_dtypes.bfloat16)
    A = np.asarray(A)
    X = np.asarray(X)
    in_maps = []
    for b in range(B):
        in_maps.append({
            "A": np.ascontiguousarray(A[b]).astype(ml_dtypes.bfloat16),
            "XT": np.ascontiguousarray(np.asarray(X[b], np.float32).T
                                       ).astype(ml_dtypes.bfloat16),
            "W": W,
            "EYE": eye,
        })
    res = run_bass_kernel_spmd(nc, in_maps, core_ids=list(range(B)),
                               trace=trace)
    S = np.stack([r["S"] for r in res.results]).astype(np.float32)
    XP = np.stack([r["XP"] for r in res.results]).astype(np.float32)
    AP = np.stack([r["AP"] for r in res.results]).astype(np.float32)
    AUX = np.stack([r["AUX"] for r in res.results]).astype(np.float64)

    sumA = AUX[:, :, 0].sum(1)
    entsum = AUX[:, :, 1].sum(1)
    gsq = AUX[:, :, 2].sum(1)
    tr = np.trace(AP.astype(np.float64), axis1=1, axis2=2)
    LP = np.sqrt(np.maximum(sumA - 2.0 * tr + gsq, 0.0))
    LP_loss = np.float32(LP.mean())
    entr_loss = np.float32(-entsum.sum() / (B * N))
    out = (XP, AP, S, LP_loss, entr_loss)
    return (out, res) if trace else out


def kernel(X, A, W_emb, W_pool):
    return run(X, A, W_emb, W_pool, trace=False)


# revision 50
# speedup vs baseline: 1.0103x; 1.0103x over previous
"""DiffPool forward on 8 Trainium2 NeuronCores — one graph per core.

Per graph b (N=2048 nodes, F=C=K=128):
    fltr = D^-1/2 (A+I) D^-1/2,  Y = X @ [W_emb|W_pool]
    [Z | logits] = fltr @ Y      (d-scaling folded into matmul operands:
                                  psum = A @ (d*Y) + I @ (d*Y); out = d * psum)
    S = softmax(logits)
    X_pooled = S^T Z,  A_pooled = S^T (A S),  G = S^T S
    ||A - S S^T||_F^2 = sum(A) - 2 tr(A_pooled) + ||G||_F^2   (exact algebra —
                         the [N,N] S S^T is never materialized)
    entr = -sum(S log(S+eps))

A is binary {0,1} so it is sent to the device as bf16 losslessly (halves HBM
traffic); all matmuls run in bf16 with fp32 PSUM accumulation. Activations are
grouped by function across tiles to avoid ACT table reloads; rowsums are split
between DVE (reduce) and ACT (copy+accumulate) halves.
"""

import os
from contextlib import ExitStack

import numpy as np
import ml_dtypes

import concourse.bass as bass
import concourse.mybir as mybir
import concourse.tile as tile
from concourse import bacc
from concourse.bass import ts
from concourse.bass_utils import run_bass_kernel_spmd

F32 = mybir.dt.float32
BF16 = mybir.dt.bfloat16
AX = mybir.AxisListType.X
AF = mybir.ActivationFunctionType
OP = mybir.AluOpType

B, N, F, K, C = 8, 2048, 128, 128, 128
NT = N // 128  # 16 row-blocks
EPS = 1e-7
# phase-B processing order: drain psum pair-banks (t, t+8) early
PROC = [0, 8, 1, 9, 2, 10, 3, 11, 4, 12, 5, 13, 6, 14, 7, 15]


def _build(phases="D"):
    nc = bacc.Bacc(None, target_bir_lowering=False)
    lvl = {"0": 0, "A": 1, "B": 2, "D": 3}[phases]
    _run_phases(nc, lvl)
    if not nc.is_finalized():
        nc.finalize()
    return nc


def _run_phases(nc, lvl):
    A_d = nc.dram_tensor("A", [N, N], BF16, kind="ExternalInput")
    XT_d = nc.dram_tensor("XT", [F, N], BF16, kind="ExternalInput")
    W_d = nc.dram_tensor("W", [F, 2 * C], BF16, kind="ExternalInput")
    EYE_d = nc.dram_tensor("EYE", [128, 128], BF16, kind="ExternalInput")

    S_d = nc.dram_tensor("S", [N, K], F32, kind="ExternalOutput")
    XP_d = nc.dram_tensor("XP", [K, C], F32, kind="ExternalOutput")
    AP_d = nc.dram_tensor("AP", [K, K], F32, kind="ExternalOutput")
    AUX_d = nc.dram_tensor("AUX", [128, 3], F32, kind="ExternalOutput")

    with tile.TileContext(nc) as tc, ExitStack() as ctx:
        per = ctx.enter_context(tc.tile_pool(name="per", bufs=1))
        tmp = ctx.enter_context(tc.tile_pool(name="tmp", bufs=3))
        sml = ctx.enter_context(tc.tile_pool(name="sml", bufs=4))
        ps = ctx.enter_context(tc.tile_pool(name="ps", bufs=1, space="PSUM"))

        # ---- phase 0: X^T, W, EYE in; Y = X @ [W_emb|W_pool] per block ----
        a16 = per.tile([128, NT, N], BF16, tag="a16")
        xt16 = per.tile([128, N], BF16, tag="xt16")
        nc.sync.dma_start(out=xt16, in_=XT_d[:, :])
        w16 = per.tile([128, 2 * C], BF16, tag="w16")
        nc.sync.dma_start(out=w16, in_=W_d[:, :])
        eye16 = per.tile([128, 128], BF16, tag="eye16")
        nc.sync.dma_start(out=eye16, in_=EYE_d[:, :])

        y32 = per.tile([128, NT, 2 * C], F32, tag="y32")
        for j in range(NT):
            yp = ps.tile([128, 2 * C], F32, tag=f"b{6 + j % 2}", name=f"yp{j}")
            nc.tensor.matmul(yp, xt16[:, ts(j, 128)], w16, start=True, stop=True)
            nc.vector.tensor_copy(out=y32[:, j, :], in_=yp)

        if lvl < 1:
            return
        # ---- phase A: stream A row-blocks; rowsum -> d; pass-1 matmuls ----
        raD = per.tile([128, NT], F32, tag="raD")
        raA = per.tile([128, NT], F32, tag="raA")
        rsum = per.tile([128, NT], F32, tag="rsum")
        sq_all = per.tile([128, NT], F32, tag="sq")
        d_all = per.tile([128, NT], F32, tag="d")
        ent_all = per.tile([128, NT], F32, tag="ent")

        accs = [ps.tile([128, 512], F32, tag=f"b{t}", name=f"acc{t}")
                for t in range(8)]

        def acc_region(i):
            return accs[i % 8][:, (i // 8) * 256:(i // 8) * 256 + 256]

        # PSUM start/stop is bank-granular: each [128,512] bank (holding the
        # accumulators for output tiles t and t+8) is ONE accumulation group
        # of 34 matmuls (2*16 A-blocks + 2 self-loop identities).
        n_mm = [0] * 8
        BANK_MMS = 34
        HALF = N // 2
        for j in range(NT):
            nc.sync.dma_start(out=a16[:, j, :], in_=A_d[ts(j, 128), :])
            # rowsum(A), split ACT/DVE (exact: binary summands into fp32)
            rscr = tmp.tile([128, 1280], BF16, tag="rscr")
            nc.scalar.activation(out=rscr, in_=a16[:, j, 0:1280],
                                 func=AF.Copy, accum_out=raA[:, j:j + 1])
            nc.vector.reduce_sum(out=raD[:, j:j + 1],
                                 in_=a16[:, j, 1280:2048], axis=AX)
            nc.vector.tensor_add(rsum[:, j:j + 1], raD[:, j:j + 1],
                                 raA[:, j:j + 1])
            # d = 1/sqrt(rowsum + 1)
            nc.scalar.activation(out=sq_all[:, j:j + 1], in_=rsum[:, j:j + 1],
                                 func=AF.Sqrt, bias=1.0, scale=1.0)
            nc.vector.reciprocal(out=d_all[:, j:j + 1], in_=sq_all[:, j:j + 1])
            # yhat_j = d_j * Y_j   (bf16)
            yh = tmp.tile([128, 2 * C], BF16, tag="yhat")
            nc.vector.tensor_scalar_mul(out=yh, in0=y32[:, j, :],
                                        scalar1=d_all[:, j:j + 1])
            # banks 6/7 last: their PSUM slots are freed by the phase-0
            # Y-psum drains, so ordering them last keeps the in-order PE
            # queue from stalling on them at the start of block 0
            for i in (0, 1, 2, 3, 4, 5, 8, 9, 10, 11, 12, 13, 6, 14, 7, 15):
                t = i % 8
                nc.tensor.matmul(acc_region(i), a16[:, j, ts(i, 128)], yh,
                                 start=(n_mm[t] == 0),
                                 stop=(n_mm[t] == BANK_MMS - 1))
                n_mm[t] += 1
                if i == j:  # self-loop: psum_i += I^T @ yhat_i
                    nc.tensor.matmul(acc_region(i), eye16, yh,
                                     start=(n_mm[t] == 0),
                                     stop=(n_mm[t] == BANK_MMS - 1))
                    n_mm[t] += 1

        if lvl < 2:
            return
        # ---- phase B: softmax / entropy / Z scaling / S out / XG matmul ----
        # Structured as per-function passes over all 16 tiles so the ACT
        # engine loads each activation table exactly once.
        s32 = per.tile([128, NT, K], F32, tag="s32")      # S in fp32 (DMA out)
        zs16 = per.tile([128, NT, 2 * C], BF16, tag="zs16")  # [Z | S] bf16
        eps_t = per.tile([128, 1], F32, tag="eps")
        nc.vector.memset(eps_t, EPS)

        sumexps = per.tile([128, NT], F32, tag="sumexps")
        rexps = per.tile([128, NT], F32, tag="rexps")

        xg = ps.tile([128, 256], F32, tag="b0")  # bank 0 after drain
        # per-tile chains (all ACT work here is Exp -> one table load);
        # consecutive tiles pipeline across DVE/ACT/PE. No max-subtraction:
        # logits = d * psum with |logit| < 1 (d <= 1/sqrt(deg) ~ 1/14), so
        # exp cannot overflow, and softmax is shift-invariant.
        for step, i in enumerate(PROC):
            # E = exp(d*logits), rowsum -> sumexp
            nc.scalar.activation(out=s32[:, i, :], in_=acc_region(i)[:, 128:256],
                                 func=AF.Exp, bias=0.0,
                                 scale=d_all[:, i:i + 1],
                                 accum_out=sumexps[:, i:i + 1])
            nc.vector.reciprocal(out=rexps[:, i:i + 1], in_=sumexps[:, i:i + 1])
            # bf16 S directly from E (shortest path to the T-pass); the fp32
            # S for DMA/entropy is produced later, off the critical path
            nc.vector.tensor_scalar_mul(out=zs16[:, i, 128:256],
                                        in0=s32[:, i, :],
                                        scalar1=rexps[:, i:i + 1])
            nc.vector.tensor_scalar_mul(out=zs16[:, i, 0:128],
                                        in0=acc_region(i)[:, 0:128],
                                        scalar1=d_all[:, i:i + 1])
            # [X_pooled | G] += S_i^T @ [Z_i | S_i]   (bf16)
            nc.tensor.matmul(xg, zs16[:, i, 128:256], zs16[:, i, :],
                             start=(step == 0), stop=(step == NT - 1))

        # XP and ||G||^2 as soon as the XG accumulation stops (overlaps C)
        aux = per.tile([128, 3], F32, tag="aux")
        xp_out = per.tile([128, 128], F32, tag="xpo")
        nc.vector.tensor_copy(out=xp_out, in_=xg[:, 0:128])
        nc.sync.dma_start(out=XP_d[:, :], in_=xp_out)
        gscr = per.tile([128, 128], F32, tag="gscr")
        nc.scalar.activation(out=gscr, in_=xg[:, 128:256], func=AF.Square,
                             accum_out=aux[:, 2:3])
        nc.vector.reduce_sum(out=aux[:, 0:1], in_=rsum, axis=AX)

        if lvl < 3:
            return
        # ---- phase C: T = A @ S (bf16), A_pooled = S^T T ----
        tps = [ps.tile([128, 512], F32, tag=f"b{1 + q}", name=f"tp{q}")
               for q in range(4)]

        def t_region(i):
            return tps[i // 4][:, (i % 4) * 128:(i % 4) * 128 + 128]

        # Bank-major: finish one tp bank's 64-matmul group, then drain it
        # (ACT bf16 copy + A_pooled matmuls) while the next bank accumulates.
        ap_ps = ps.tile([128, 128], F32, tag="b5")
        for q in range(4):
            for idx, j in enumerate(PROC):
                for r in range(4):
                    i = 4 * q + r
                    nc.tensor.matmul(t_region(i), a16[:, j, ts(i, 128)],
                                     zs16[:, j, 128:256],
                                     start=(idx == 0 and r == 0),
                                     stop=(idx == NT - 1 and r == 3))
            for r in range(4):
                i = 4 * q + r
                t16 = tmp.tile([128, 128], BF16, tag="t16", bufs=4,
                               name=f"t16_{i}")
                nc.vector.tensor_copy(out=t16, in_=t_region(i))
                nc.tensor.matmul(ap_ps, zs16[:, i, 128:256], t16,
                                 start=(i == 0), stop=(i == NT - 1))

        # S out + entropy (off the critical path; only feeds S_d and AUX)
        for i in PROC:  # fp32 S = E/sumexp, in place over E
            nc.vector.tensor_scalar_mul(out=s32[:, i, :], in0=s32[:, i, :],
                                        scalar1=rexps[:, i:i + 1])
        for i in PROC:
            nc.sync.dma_start(out=S_d[ts(i, 128), :], in_=s32[:, i, :])
        for i in PROC:  # ACT: ln(S+eps); DVE: entropy partial
            l32 = tmp.tile([128, 128], F32, tag="l32")
            nc.scalar.activation(out=l32, in_=s32[:, i, :], func=AF.Ln,
                                 bias=eps_t, scale=1.0)
            escr = tmp.tile([128, 128], F32, tag="escr")
            nc.vector.tensor_mul(escr, s32[:, i, :], l32)
            nc.vector.reduce_sum(out=ent_all[:, i:i + 1], in_=escr,
                                 axis=AX)

        # ---- phase D: remaining outputs ----
        ap_out = per.tile([128, 128], F32, tag="apo")
        nc.vector.tensor_copy(out=ap_out, in_=ap_ps)
        nc.sync.dma_start(out=AP_d[:, :], in_=ap_out)
        nc.vector.reduce_sum(out=aux[:, 1:2], in_=ent_all, axis=AX)
        nc.sync.dma_start(out=AUX_d[:, :], in_=aux)


_NC = None


def _get_nc():
    global _NC
    if _NC is None:
        _NC = _build(os.environ.get("DIFFPOOL_PHASES", "D"))
    return _NC


def run(X, A, W_emb, W_pool, trace=False):
    nc = _get_nc()
    W = np.concatenate([np.asarray(W_emb, np.float32),
                        np.asarray(W_pool, np.float32)], axis=1)
    W = W.astype(ml_dtypes.bfloat16)
    eye = np.eye(128, dtype=np.float32).astype(ml_dtypes.bfloat16)
    A = np.asarray(A)
    X = np.asarray(X)
    in_maps = []
    for b in range(B):
        in_maps.append({
            "A": np.ascontiguousarray(A[b]).astype(ml_dtypes.bfloat16),
            "XT": np.ascontiguousarray(np.asarray(X[b], np.float32).T
                                       ).astype(ml_dtypes.bfloat16),
            "W": W,
            "EYE": eye,
        })
    res = run_bass_kernel_spmd(nc, in_maps, core_ids=list(range(B)),
                               trace=trace)
    S = np.stack([r["S"] for r in res.results]).astype(np.float32)
    XP = np.stack([r["XP"] for r in res.results]).astype(np.float32)
    AP = np.stack([r["AP"] for r in res.results]).astype(np.float32)
    AUX = np.stack([r["AUX"] for r in res.results]).astype(np.float64)

    sumA = AUX[:, :, 0].sum(1)
    entsum = AUX[:, :, 1].sum(1)
    gsq = AUX[:, :, 2].sum(1)
    tr = np.trace(AP.astype(np.float64), axis1=1, axis2=2)
    LP = np.sqrt(np.maximum(sumA - 2.0 * tr + gsq, 0.0))
    LP_loss = np.float32(LP.mean())
    entr_loss = np.float32(-entsum.sum() / (B * N))
    out = (XP, AP, S, LP_loss, entr_loss)
    return (out, res) if trace else out


def kernel(X, A, W_emb, W_pool):
    return run(X, A, W_emb, W_pool, trace=False)


# revision 51
# speedup vs baseline: 1.0358x; 1.0253x over previous
"""DiffPool forward on 8 Trainium2 NeuronCores — one graph per core.

Per graph b (N=2048 nodes, F=C=K=128):
    fltr = D^-1/2 (A+I) D^-1/2,  Y = X @ [W_emb|W_pool]
    [Z | logits] = fltr @ Y      (d-scaling folded into matmul operands:
                                  psum = A @ (d*Y) + I @ (d*Y); out = d * psum)
    S = softmax(logits)
    X_pooled = S^T Z,  A_pooled = S^T (A S),  G = S^T S
    ||A - S S^T||_F^2 = sum(A) - 2 tr(A_pooled) + ||G||_F^2   (exact algebra —
                         the [N,N] S S^T is never materialized)
    entr = -sum(S log(S+eps))

A is binary {0,1} so it is sent to the device as bf16 losslessly (halves HBM
traffic); all matmuls run in bf16 with fp32 PSUM accumulation. Activations are
grouped by function across tiles to avoid ACT table reloads; rowsums are split
between DVE (reduce) and ACT (copy+accumulate) halves.
"""

import os
from contextlib import ExitStack

import numpy as np
import ml_dtypes

import concourse.bass as bass
import concourse.mybir as mybir
import concourse.tile as tile
from concourse import bacc
from concourse.bass import ts
from concourse.bass_utils import run_bass_kernel_spmd

F32 = mybir.dt.float32
BF16 = mybir.dt.bfloat16
AX = mybir.AxisListType.X
AF = mybir.ActivationFunctionType
OP = mybir.AluOpType

B, N, F, K, C = 8, 2048, 128, 128, 128
NT = N // 128  # 16 row-blocks
EPS = 1e-7
# phase-B processing order: drain psum pair-banks (t, t+8) early
PROC = [0, 8, 1, 9, 2, 10, 3, 11, 4, 12, 5, 13, 6, 14, 7, 15]


def _build(phases="D"):
    nc = bacc.Bacc(None, target_bir_lowering=False)
    lvl = {"0": 0, "A": 1, "B": 2, "D": 3}[phases]
    _run_phases(nc, lvl)
    if not nc.is_finalized():
        nc.finalize()
    return nc


def _run_phases(nc, lvl):
    A_d = nc.dram_tensor("A", [N, N], BF16, kind="ExternalInput")
    XT_d = nc.dram_tensor("XT", [F, N], BF16, kind="ExternalInput")
    W_d = nc.dram_tensor("W", [F, 2 * C], BF16, kind="ExternalInput")
    EYE_d = nc.dram_tensor("EYE", [128, 128], BF16, kind="ExternalInput")

    S_d = nc.dram_tensor("S", [N, K], F32, kind="ExternalOutput")
    XP_d = nc.dram_tensor("XP", [K, C], F32, kind="ExternalOutput")
    AP_d = nc.dram_tensor("AP", [K, K], F32, kind="ExternalOutput")
    AUX_d = nc.dram_tensor("AUX", [128, 3], F32, kind="ExternalOutput")

    with tile.TileContext(nc) as tc, ExitStack() as ctx:
        per = ctx.enter_context(tc.tile_pool(name="per", bufs=1))
        tmp = ctx.enter_context(tc.tile_pool(name="tmp", bufs=3))
        sml = ctx.enter_context(tc.tile_pool(name="sml", bufs=4))
        ps = ctx.enter_context(tc.tile_pool(name="ps", bufs=1, space="PSUM"))

        # ---- phase 0: X^T, W, EYE in; Y = X @ [W_emb|W_pool] per block ----
        a16 = per.tile([128, NT, N], BF16, tag="a16")
        xt16 = per.tile([128, N], BF16, tag="xt16")
        nc.sync.dma_start(out=xt16, in_=XT_d[:, :])
        w16 = per.tile([128, 2 * C], BF16, tag="w16")
        nc.sync.dma_start(out=w16, in_=W_d[:, :])
        eye16 = per.tile([128, 128], BF16, tag="eye16")
        nc.sync.dma_start(out=eye16, in_=EYE_d[:, :])

        y32 = per.tile([128, NT, 2 * C], F32, tag="y32")
        for j in range(NT):
            yp = ps.tile([128, 2 * C], F32, tag=f"b{6 + j % 2}", name=f"yp{j}")
            nc.tensor.matmul(yp, xt16[:, ts(j, 128)], w16, start=True, stop=True)
            nc.vector.tensor_copy(out=y32[:, j, :], in_=yp)

        if lvl < 1:
            return
        # ---- phase A: stream A row-blocks; rowsum -> d; pass-1 matmuls ----
        raD = per.tile([128, NT], F32, tag="raD")
        raA = per.tile([128, NT], F32, tag="raA")
        rsum = per.tile([128, NT], F32, tag="rsum")
        sq_all = per.tile([128, NT], F32, tag="sq")
        d_all = per.tile([128, NT], F32, tag="d")
        ent_all = per.tile([128, NT], F32, tag="ent")

        accs = [ps.tile([128, 512], F32, tag=f"b{t}", name=f"acc{t}")
                for t in range(8)]

        def acc_region(i):
            return accs[i % 8][:, (i // 8) * 256:(i // 8) * 256 + 256]

        # PSUM start/stop is bank-granular: each [128,512] bank (holding the
        # accumulators for output tiles t and t+8) is ONE accumulation group
        # of 34 matmuls (2*16 A-blocks + 2 self-loop identities).
        n_mm = [0] * 8
        BANK_MMS = 34
        HALF = N // 2
        for j in range(NT):
            nc.sync.dma_start(out=a16[:, j, :], in_=A_d[ts(j, 128), :])
            # rowsum(A), split ACT/DVE (exact: binary summands into fp32).
            # Early blocks lean on ACT so DVE can clear the Y-copy burst.
            cut = 1792 if j < 5 else 1280
            rscr = tmp.tile([128, 1792], BF16, tag="rscr")
            nc.scalar.activation(out=rscr[:, 0:cut], in_=a16[:, j, 0:cut],
                                 func=AF.Copy, accum_out=raA[:, j:j + 1])
            nc.vector.reduce_sum(out=raD[:, j:j + 1],
                                 in_=a16[:, j, cut:2048], axis=AX)
            nc.vector.tensor_add(rsum[:, j:j + 1], raD[:, j:j + 1],
                                 raA[:, j:j + 1])
            # d = 1/sqrt(rowsum + 1)
            nc.scalar.activation(out=sq_all[:, j:j + 1], in_=rsum[:, j:j + 1],
                                 func=AF.Sqrt, bias=1.0, scale=1.0)
            nc.vector.reciprocal(out=d_all[:, j:j + 1], in_=sq_all[:, j:j + 1])
            # yhat_j = d_j * Y_j   (bf16)
            yh = tmp.tile([128, 2 * C], BF16, tag="yhat")
            nc.vector.tensor_scalar_mul(out=yh, in0=y32[:, j, :],
                                        scalar1=d_all[:, j:j + 1])
            # banks 6/7 last: their PSUM slots are freed by the phase-0
            # Y-psum drains, so ordering them last keeps the in-order PE
            # queue from stalling on them at the start of block 0
            for i in (0, 1, 2, 3, 4, 5, 8, 9, 10, 11, 12, 13, 6, 14, 7, 15):
                t = i % 8
                nc.tensor.matmul(acc_region(i), a16[:, j, ts(i, 128)], yh,
                                 start=(n_mm[t] == 0),
                                 stop=(n_mm[t] == BANK_MMS - 1))
                n_mm[t] += 1
                if i == j:  # self-loop: psum_i += I^T @ yhat_i
                    nc.tensor.matmul(acc_region(i), eye16, yh,
                                     start=(n_mm[t] == 0),
                                     stop=(n_mm[t] == BANK_MMS - 1))
                    n_mm[t] += 1

        if lvl < 2:
            return
        # ---- phase B: softmax / entropy / Z scaling / S out / XG matmul ----
        # Structured as per-function passes over all 16 tiles so the ACT
        # engine loads each activation table exactly once.
        s32 = per.tile([128, NT, K], F32, tag="s32")      # S in fp32 (DMA out)
        zs16 = per.tile([128, NT, 2 * C], BF16, tag="zs16")  # [Z | S] bf16
        eps_t = per.tile([128, 1], F32, tag="eps")
        nc.vector.memset(eps_t, EPS)

        sumexps = per.tile([128, NT], F32, tag="sumexps")
        rexps = per.tile([128, NT], F32, tag="rexps")

        xg = ps.tile([128, 256], F32, tag="b0")  # bank 0 after drain
        # per-tile chains (all ACT work here is Exp -> one table load);
        # consecutive tiles pipeline across DVE/ACT/PE. No max-subtraction:
        # logits = d * psum with |logit| < 1 (d <= 1/sqrt(deg) ~ 1/14), so
        # exp cannot overflow, and softmax is shift-invariant.
        for step, i in enumerate(PROC):
            # E = exp(d*logits), rowsum -> sumexp
            nc.scalar.activation(out=s32[:, i, :], in_=acc_region(i)[:, 128:256],
                                 func=AF.Exp, bias=0.0,
                                 scale=d_all[:, i:i + 1],
                                 accum_out=sumexps[:, i:i + 1])
            nc.vector.reciprocal(out=rexps[:, i:i + 1], in_=sumexps[:, i:i + 1])
            # bf16 S directly from E (shortest path to the T-pass); the fp32
            # S for DMA/entropy is produced later, off the critical path
            nc.vector.tensor_scalar_mul(out=zs16[:, i, 128:256],
                                        in0=s32[:, i, :],
                                        scalar1=rexps[:, i:i + 1])
            nc.vector.tensor_scalar_mul(out=zs16[:, i, 0:128],
                                        in0=acc_region(i)[:, 0:128],
                                        scalar1=d_all[:, i:i + 1])
            # [X_pooled | G] += S_i^T @ [Z_i | S_i]   (bf16)
            nc.tensor.matmul(xg, zs16[:, i, 128:256], zs16[:, i, :],
                             start=(step == 0), stop=(step == NT - 1))

        # XP and ||G||^2 as soon as the XG accumulation stops (overlaps C)
        aux = per.tile([128, 3], F32, tag="aux")
        xp_out = per.tile([128, 128], F32, tag="xpo")
        nc.vector.tensor_copy(out=xp_out, in_=xg[:, 0:128])
        nc.sync.dma_start(out=XP_d[:, :], in_=xp_out)
        gscr = per.tile([128, 128], F32, tag="gscr")
        nc.scalar.activation(out=gscr, in_=xg[:, 128:256], func=AF.Square,
                             accum_out=aux[:, 2:3])
        nc.vector.reduce_sum(out=aux[:, 0:1], in_=rsum, axis=AX)

        if lvl < 3:
            return
        # ---- phase C: T = A @ S (bf16), A_pooled = S^T T ----
        tps = [ps.tile([128, 512], F32, tag=f"b{1 + q}", name=f"tp{q}")
               for q in range(4)]

        def t_region(i):
            return tps[i // 4][:, (i % 4) * 128:(i % 4) * 128 + 128]

        # Bank-major: finish one tp bank's 64-matmul group, then drain it
        # (ACT bf16 copy + A_pooled matmuls) while the next bank accumulates.
        ap_ps = ps.tile([128, 128], F32, tag="b5")
        for q in range(4):
            for idx, j in enumerate(PROC):
                for r in range(4):
                    i = 4 * q + r
                    nc.tensor.matmul(t_region(i), a16[:, j, ts(i, 128)],
                                     zs16[:, j, 128:256],
                                     start=(idx == 0 and r == 0),
                                     stop=(idx == NT - 1 and r == 3))
            for r in range(4):
                i = 4 * q + r
                t16 = tmp.tile([128, 128], BF16, tag="t16", bufs=4,
                               name=f"t16_{i}")
                nc.vector.tensor_copy(out=t16, in_=t_region(i))
                nc.tensor.matmul(ap_ps, zs16[:, i, 128:256], t16,
                                 start=(i == 0), stop=(i == NT - 1))

        # S out + entropy (off the critical path; only feeds S_d and AUX)
        for i in PROC:  # fp32 S = E/sumexp, in place over E
            nc.vector.tensor_scalar_mul(out=s32[:, i, :], in0=s32[:, i, :],
                                        scalar1=rexps[:, i:i + 1])
        for i in PROC:
            nc.sync.dma_start(out=S_d[ts(i, 128), :], in_=s32[:, i, :])
        for i in PROC:  # ACT: ln(S+eps); DVE: entropy partial
            l32 = tmp.tile([128, 128], F32, tag="l32")
            nc.scalar.activation(out=l32, in_=s32[:, i, :], func=AF.Ln,
                                 bias=eps_t, scale=1.0)
            escr = tmp.tile([128, 128], F32, tag="escr")
            nc.vector.tensor_mul(escr, s32[:, i, :], l32)
            nc.vector.reduce_sum(out=ent_all[:, i:i + 1], in_=escr,
                                 axis=AX)

        # ---- phase D: remaining outputs ----
        ap_out = per.tile([128, 128], F32, tag="apo")
        nc.vector.tensor_copy(out=ap_out, in_=ap_ps)
        nc.sync.dma_start(out=AP_d[:, :], in_=ap_out)
        nc.vector.reduce_sum(out=aux[:, 1:2], in_=ent_all, axis=AX)
        nc.sync.dma_start(out=AUX_d[:, :], in_=aux)


_NC = None


def _get_nc():
    global _NC
    if _NC is None:
        _NC = _build(os.environ.get("DIFFPOOL_PHASES", "D"))
    return _NC


def run(X, A, W_emb, W_pool, trace=False):
    nc = _get_nc()
    W = np.concatenate([np.asarray(W_emb, np.float32),
                        np.asarray(W_pool, np.float32)], axis=1)
    W = W.astype(ml_dtypes.bfloat16)
    eye = np.eye(128, dtype=np.float32).astype(ml_dtypes.bfloat16)
    A = np.asarray(A)
    X = np.asarray(X)
    in_maps = []
    for b in range(B):
        in_maps.append({
            "A": np.ascontiguousarray(A[b]).astype(ml_dtypes.bfloat16),
            "XT": np.ascontiguousarray(np.asarray(X[b], np.float32).T
                                       ).astype(ml_dtypes.bfloat16),
            "W": W,
            "EYE": eye,
        })
    res = run_bass_kernel_spmd(nc, in_maps, core_ids=list(range(B)),
                               trace=trace)
    S = np.stack([r["S"] for r in res.results]).astype(np.float32)
    XP = np.stack([r["XP"] for r in res.results]).astype(np.float32)
    AP = np.stack([r["AP"] for r in res.results]).astype(np.float32)
    AUX = np.stack([r["AUX"] for r in res.results]).astype(np.float64)

    sumA = AUX[:, :, 0].sum(1)
    entsum = AUX[:, :, 1].sum(1)
    gsq = AUX[:, :, 2].sum(1)
    tr = np.trace(AP.astype(np.float64), axis1=1, axis2=2)
    LP = np.sqrt(np.maximum(sumA - 2.0 * tr + gsq, 0.0))
    LP_loss = np.float32(LP.mean())
    entr_loss = np.float32(-entsum.sum() / (B * N))
    out = (XP, AP, S, LP_loss, entr_loss)
    return (out, res) if trace else out


def kernel(X, A, W_emb, W_pool):
    return run(X, A, W_emb, W_pool, trace=False)


# revision 52
# speedup vs baseline: 1.0508x; 1.0144x over previous
"""DiffPool forward on 8 Trainium2 NeuronCores — one graph per core.

Per graph b (N=2048 nodes, F=C=K=128):
    fltr = D^-1/2 (A+I) D^-1/2,  Y = X @ [W_emb|W_pool]
    [Z | logits] = fltr @ Y      (d-scaling folded into matmul operands:
                                  psum = A @ (d*Y) + I @ (d*Y); out = d * psum)
    S = softmax(logits)
    X_pooled = S^T Z,  A_pooled = S^T (A S),  G = S^T S
    ||A - S S^T||_F^2 = sum(A) - 2 tr(A_pooled) + ||G||_F^2   (exact algebra —
                         the [N,N] S S^T is never materialized)
    entr = -sum(S log(S+eps))

A is binary {0,1} so it is sent to the device as bf16 losslessly (halves HBM
traffic); all matmuls run in bf16 with fp32 PSUM accumulation. Activations are
grouped by function across tiles to avoid ACT table reloads; rowsums are split
between DVE (reduce) and ACT (copy+accumulate) halves.
"""

import os
from contextlib import ExitStack

import numpy as np
import ml_dtypes

import concourse.bass as bass
import concourse.mybir as mybir
import concourse.tile as tile
from concourse import bacc
from concourse.bass import ts
from concourse.bass_utils import run_bass_kernel_spmd

F32 = mybir.dt.float32
BF16 = mybir.dt.bfloat16
AX = mybir.AxisListType.X
AF = mybir.ActivationFunctionType
OP = mybir.AluOpType

B, N, F, K, C = 8, 2048, 128, 128, 128
NT = N // 128  # 16 row-blocks
EPS = 1e-7
# phase-B processing order: drain psum pair-banks (t, t+8) early
PROC = [0, 8, 1, 9, 2, 10, 3, 11, 4, 12, 5, 13, 6, 14, 7, 15]


def _build(phases="D"):
    nc = bacc.Bacc(None, target_bir_lowering=False)
    lvl = {"0": 0, "A": 1, "B": 2, "D": 3}[phases]
    _run_phases(nc, lvl)
    if not nc.is_finalized():
        nc.finalize()
    return nc


def _run_phases(nc, lvl):
    A_d = nc.dram_tensor("A", [N, N], BF16, kind="ExternalInput")
    XT_d = nc.dram_tensor("XT", [F, N], BF16, kind="ExternalInput")
    W_d = nc.dram_tensor("W", [F, 2 * C], BF16, kind="ExternalInput")
    EYE_d = nc.dram_tensor("EYE", [128, 128], BF16, kind="ExternalInput")

    S_d = nc.dram_tensor("S", [N, K], F32, kind="ExternalOutput")
    XP_d = nc.dram_tensor("XP", [K, C], F32, kind="ExternalOutput")
    AP_d = nc.dram_tensor("AP", [K, K], F32, kind="ExternalOutput")
    AUX_d = nc.dram_tensor("AUX", [128, 3], F32, kind="ExternalOutput")

    with tile.TileContext(nc) as tc, ExitStack() as ctx:
        per = ctx.enter_context(tc.tile_pool(name="per", bufs=1))
        tmp = ctx.enter_context(tc.tile_pool(name="tmp", bufs=3))
        sml = ctx.enter_context(tc.tile_pool(name="sml", bufs=4))
        ps = ctx.enter_context(tc.tile_pool(name="ps", bufs=1, space="PSUM"))

        # ---- phase 0: X^T, W, EYE in; Y = X @ [W_emb|W_pool] per block ----
        a16 = per.tile([128, NT, N], BF16, tag="a16")
        xt16 = per.tile([128, N], BF16, tag="xt16")
        nc.sync.dma_start(out=xt16, in_=XT_d[:, :])
        w16 = per.tile([128, 2 * C], BF16, tag="w16")
        nc.sync.dma_start(out=w16, in_=W_d[:, :])
        eye16 = per.tile([128, 128], BF16, tag="eye16")
        nc.sync.dma_start(out=eye16, in_=EYE_d[:, :])

        y32 = per.tile([128, NT, 2 * C], F32, tag="y32")
        for j in range(NT):
            yp = ps.tile([128, 2 * C], F32, tag=f"b{6 + j % 2}", name=f"yp{j}")
            nc.tensor.matmul(yp, xt16[:, ts(j, 128)], w16, start=True, stop=True)
            nc.vector.tensor_copy(out=y32[:, j, :], in_=yp)

        if lvl < 1:
            return
        # ---- phase A: stream A row-blocks; rowsum -> d; pass-1 matmuls ----
        raD = per.tile([128, NT], F32, tag="raD")
        raA = per.tile([128, NT], F32, tag="raA")
        rsum = per.tile([128, NT], F32, tag="rsum")
        sq_all = per.tile([128, NT], F32, tag="sq")
        d_all = per.tile([128, NT], F32, tag="d")
        ent_all = per.tile([128, NT], F32, tag="ent")

        accs = [ps.tile([128, 512], F32, tag=f"b{t}", name=f"acc{t}")
                for t in range(8)]

        def acc_region(i):
            return accs[i % 8][:, (i // 8) * 256:(i // 8) * 256 + 256]

        # PSUM start/stop is bank-granular: each [128,512] bank (holding the
        # accumulators for output tiles t and t+8) is ONE accumulation group
        # of 34 matmuls (2*16 A-blocks + 2 self-loop identities).
        n_mm = [0] * 8
        BANK_MMS = 34
        HALF = N // 2
        for j in range(NT):
            nc.sync.dma_start(out=a16[:, j, :], in_=A_d[ts(j, 128), :])
            # rowsum(A), split ACT/DVE (exact: binary summands into fp32).
            # Early blocks go fully/mostly to ACT so DVE can clear the
            # Y-copy burst that gates the first pass-1 matmuls.
            cut = 2048 if j < 3 else (1792 if j < 6 else 1280)
            rscr = tmp.tile([128, 2048], BF16, tag="rscr")
            nc.scalar.activation(out=rscr[:, 0:cut], in_=a16[:, j, 0:cut],
                                 func=AF.Copy,
                                 accum_out=(rsum if cut == 2048 else
                                            raA)[:, j:j + 1])
            if cut < 2048:
                nc.vector.reduce_sum(out=raD[:, j:j + 1],
                                     in_=a16[:, j, cut:2048], axis=AX)
                nc.vector.tensor_add(rsum[:, j:j + 1], raD[:, j:j + 1],
                                     raA[:, j:j + 1])
            # d = 1/sqrt(rowsum + 1)
            nc.scalar.activation(out=sq_all[:, j:j + 1], in_=rsum[:, j:j + 1],
                                 func=AF.Sqrt, bias=1.0, scale=1.0)
            nc.vector.reciprocal(out=d_all[:, j:j + 1], in_=sq_all[:, j:j + 1])
            # yhat_j = d_j * Y_j   (bf16)
            yh = tmp.tile([128, 2 * C], BF16, tag="yhat")
            nc.vector.tensor_scalar_mul(out=yh, in0=y32[:, j, :],
                                        scalar1=d_all[:, j:j + 1])
            # banks 6/7 last: their PSUM slots are freed by the phase-0
            # Y-psum drains, so ordering them last keeps the in-order PE
            # queue from stalling on them at the start of block 0
            for i in (0, 1, 2, 3, 4, 5, 8, 9, 10, 11, 12, 13, 6, 14, 7, 15):
                t = i % 8
                nc.tensor.matmul(acc_region(i), a16[:, j, ts(i, 128)], yh,
                                 start=(n_mm[t] == 0),
                                 stop=(n_mm[t] == BANK_MMS - 1))
                n_mm[t] += 1
                if i == j:  # self-loop: psum_i += I^T @ yhat_i
                    nc.tensor.matmul(acc_region(i), eye16, yh,
                                     start=(n_mm[t] == 0),
                                     stop=(n_mm[t] == BANK_MMS - 1))
                    n_mm[t] += 1

        if lvl < 2:
            return
        # ---- phase B: softmax / entropy / Z scaling / S out / XG matmul ----
        # Structured as per-function passes over all 16 tiles so the ACT
        # engine loads each activation table exactly once.
        s32 = per.tile([128, NT, K], F32, tag="s32")      # S in fp32 (DMA out)
        zs16 = per.tile([128, NT, 2 * C], BF16, tag="zs16")  # [Z | S] bf16
        eps_t = per.tile([128, 1], F32, tag="eps")
        nc.vector.memset(eps_t, EPS)

        sumexps = per.tile([128, NT], F32, tag="sumexps")
        rexps = per.tile([128, NT], F32, tag="rexps")

        xg = ps.tile([128, 256], F32, tag="b0")  # bank 0 after drain
        # per-tile chains (all ACT work here is Exp -> one table load);
        # consecutive tiles pipeline across DVE/ACT/PE. No max-subtraction:
        # logits = d * psum with |logit| < 1 (d <= 1/sqrt(deg) ~ 1/14), so
        # exp cannot overflow, and softmax is shift-invariant.
        for step, i in enumerate(PROC):
            # E = exp(d*logits), rowsum -> sumexp
            nc.scalar.activation(out=s32[:, i, :], in_=acc_region(i)[:, 128:256],
                                 func=AF.Exp, bias=0.0,
                                 scale=d_all[:, i:i + 1],
                                 accum_out=sumexps[:, i:i + 1])
            nc.vector.reciprocal(out=rexps[:, i:i + 1], in_=sumexps[:, i:i + 1])
            # bf16 S directly from E (shortest path to the T-pass); the fp32
            # S for DMA/entropy is produced later, off the critical path
            nc.vector.tensor_scalar_mul(out=zs16[:, i, 128:256],
                                        in0=s32[:, i, :],
                                        scalar1=rexps[:, i:i + 1])
            nc.vector.tensor_scalar_mul(out=zs16[:, i, 0:128],
                                        in0=acc_region(i)[:, 0:128],
                                        scalar1=d_all[:, i:i + 1])
            # [X_pooled | G] += S_i^T @ [Z_i | S_i]   (bf16)
            nc.tensor.matmul(xg, zs16[:, i, 128:256], zs16[:, i, :],
                             start=(step == 0), stop=(step == NT - 1))

        # XP and ||G||^2 as soon as the XG accumulation stops (overlaps C)
        aux = per.tile([128, 3], F32, tag="aux")
        xp_out = per.tile([128, 128], F32, tag="xpo")
        nc.vector.tensor_copy(out=xp_out, in_=xg[:, 0:128])
        nc.sync.dma_start(out=XP_d[:, :], in_=xp_out)
        gscr = per.tile([128, 128], F32, tag="gscr")
        nc.scalar.activation(out=gscr, in_=xg[:, 128:256], func=AF.Square,
                             accum_out=aux[:, 2:3])
        nc.vector.reduce_sum(out=aux[:, 0:1], in_=rsum, axis=AX)

        if lvl < 3:
            return
        # ---- phase C: T = A @ S (bf16), A_pooled = S^T T ----
        tps = [ps.tile([128, 512], F32, tag=f"b{1 + q}", name=f"tp{q}")
               for q in range(4)]

        def t_region(i):
            return tps[i // 4][:, (i % 4) * 128:(i % 4) * 128 + 128]

        # Bank-major: finish one tp bank's 64-matmul group, then drain it
        # (ACT bf16 copy + A_pooled matmuls) while the next bank accumulates.
        ap_ps = ps.tile([128, 128], F32, tag="b5")
        for q in range(4):
            for idx, j in enumerate(PROC):
                for r in range(4):
                    i = 4 * q + r
                    nc.tensor.matmul(t_region(i), a16[:, j, ts(i, 128)],
                                     zs16[:, j, 128:256],
                                     start=(idx == 0 and r == 0),
                                     stop=(idx == NT - 1 and r == 3))
            for r in range(4):
                i = 4 * q + r
                t16 = tmp.tile([128, 128], BF16, tag="t16", bufs=4,
                               name=f"t16_{i}")
                nc.vector.tensor_copy(out=t16, in_=t_region(i))
                nc.tensor.matmul(ap_ps, zs16[:, i, 128:256], t16,
                                 start=(i == 0), stop=(i == NT - 1))

        # S out + entropy (off the critical path; only feeds S_d and AUX)
        for i in PROC:  # fp32 S = E/sumexp, in place over E
            nc.vector.tensor_scalar_mul(out=s32[:, i, :], in0=s32[:, i, :],
                                        scalar1=rexps[:, i:i + 1])
        for i in PROC:
            nc.sync.dma_start(out=S_d[ts(i, 128), :], in_=s32[:, i, :])
        for i in PROC:  # ACT: ln(S+eps); DVE: entropy partial
            l32 = tmp.tile([128, 128], F32, tag="l32")
            nc.scalar.activation(out=l32, in_=s32[:, i, :], func=AF.Ln,
                                 bias=eps_t, scale=1.0)
            escr = tmp.tile([128, 128], F32, tag="escr")
            nc.vector.tensor_mul(escr, s32[:, i, :], l32)
            nc.vector.reduce_sum(out=ent_all[:, i:i + 1], in_=escr,
                                 axis=AX)

        # ---- phase D: remaining outputs ----
        ap_out = per.tile([128, 128], F32, tag="apo")
        nc.vector.tensor_copy(out=ap_out, in_=ap_ps)
        nc.sync.dma_start(out=AP_d[:, :], in_=ap_out)
        nc.vector.reduce_sum(out=aux[:, 1:2], in_=ent_all, axis=AX)
        nc.sync.dma_start(out=AUX_d[:, :], in_=aux)


_NC = None


def _get_nc():
    global _NC
    if _NC is None:
        _NC = _build(os.environ.get("DIFFPOOL_PHASES", "D"))
    return _NC


def run(X, A, W_emb, W_pool, trace=False):
    nc = _get_nc()
    W = np.concatenate([np.asarray(W_emb, np.float32),
                        np.asarray(W_pool, np.float32)], axis=1)
    W = W.astype(ml_dtypes.bfloat16)
    eye = np.eye(128, dtype=np.float32).astype(ml_dtypes.bfloat16)
    A = np.asarray(A)
    X = np.asarray(X)
    in_maps = []
    for b in range(B):
        in_maps.append({
            "A": np.ascontiguousarray(A[b]).astype(ml_dtypes.bfloat16),
            "XT": np.ascontiguousarray(np.asarray(X[b], np.float32).T
                                       ).astype(ml_dtypes.bfloat16),
            "W": W,
            "EYE": eye,
        })
    res = run_bass_kernel_spmd(nc, in_maps, core_ids=list(range(B)),
                               trace=trace)
    S = np.stack([r["S"] for r in res.results]).astype(np.float32)
    XP = np.stack([r["XP"] for r in res.results]).astype(np.float32)
    AP = np.stack([r["AP"] for r in res.results]).astype(np.float32)
    AUX = np.stack([r["AUX"] for r in res.results]).astype(np.float64)

    sumA = AUX[:, :, 0].sum(1)
    entsum = AUX[:, :, 1].sum(1)
    gsq = AUX[:, :, 2].sum(1)
    tr = np.trace(AP.astype(np.float64), axis1=1, axis2=2)
    LP = np.sqrt(np.maximum(sumA - 2.0 * tr + gsq, 0.0))
    LP_loss = np.float32(LP.mean())
    entr_loss = np.float32(-entsum.sum() / (B * N))
    out = (XP, AP, S, LP_loss, entr_loss)
    return (out, res) if trace else out


def kernel(X, A, W_emb, W_pool):
    return run(X, A, W_emb, W_pool, trace=False)
